# revision 11
# baseline (speedup 1.0000x reference)
"""AttnBlock (channel attention over 64x64 maps) for Trainium2 — Gram form.

Data-parallel over batch: 16 batches, 2 per core across 8 NeuronCores.
Per batch [C=512, N=4096], with hn = A.x + B (GroupNorm folded to affine):
  scores = q k^T = (WqA) XX (WkA)^T + u bk'^T + bq' (w + N bk')^T,  XX = x x^T
  attn   = softmax(scores * C^-0.5)  (no max-sub; e kept in bf16)
  out    = Wo attn v = FWt^T (A.x) + (F bv')1^T, F = Wo D_rinv E, FWt = Wv^T F^T
  y      = x + out + bo
So the N-wide work is only XX (1 unit) and the final FWt^T @ xA (1 unit);
q/k/v projections and the output projection collapse into C x C x C matmuls
(0.125 unit each). ~2.5 units/batch vs 6 for the direct form.
Matmuls run fp16 (weights/x/XX/M1/Ft/FWt) or bf16 (e, rinv-scaled Wo — range),
stats/softmax/residual fp32.
"""

import sys

if "/opt/trn_rl_repo" not in sys.path:
    sys.path.insert(0, "/opt/trn_rl_repo")

import numpy as np

C = 512          # channels
N = 4096         # pixels (64*64)
BB = 2           # batches per core
P = 128          # partitions
CB = C // P      # 4 channel blocks
NT = N // P      # 32 pixel tiles of 128 (Gram phase)
NTH = NT // 2    # half split of pixel tiles (two xt tiles => earlier prefetch)
NSL = 512        # pixel slice width (epilogue)
NS = N // NSL    # 8 pixel slices
GROUPS = 32
EPS = 1e-6
SCALE = float(C) ** -0.5

_NC_CACHE = {}
LAST_RESULT = None


def _build_nc():
    import concourse.bacc as bacc
    import concourse.tile as tile
    from concourse import mybir
    from concourse.bass import ts

    F32 = mybir.dt.float32
    BF16 = mybir.dt.bfloat16
    FP16 = mybir.dt.float16
    AF = mybir.ActivationFunctionType
    OP = mybir.AluOpType

    nc = bacc.Bacc(None, target_bir_lowering=False, num_swdge_queues=4)

    xs_d = nc.dram_tensor("xs", [BB, C, N], BF16, kind="ExternalInput")
    xt_d = nc.dram_tensor("xt", [BB, N, C], FP16, kind="ExternalInput")
    wqt_d = nc.dram_tensor("wqt", [C, C], FP16, kind="ExternalInput")
    wkt_d = nc.dram_tensor("wkt", [C, C], FP16, kind="ExternalInput")
    wvt_d = nc.dram_tensor("wvt", [C, C], FP16, kind="ExternalInput")
    wvnt_d = nc.dram_tensor("wvnt", [C, C], FP16, kind="ExternalInput")
    wot_d = nc.dram_tensor("wot", [C, C], FP16, kind="ExternalInput")
    bq_d = nc.dram_tensor("bq", [C], F32, kind="ExternalInput")
    bk_d = nc.dram_tensor("bk", [C], F32, kind="ExternalInput")
    bv_d = nc.dram_tensor("bv", [C], F32, kind="ExternalInput")
    bo_d = nc.dram_tensor("bo", [C], F32, kind="ExternalInput")
    gamma_d = nc.dram_tensor("gamma", [C], F32, kind="ExternalInput")
    beta_d = nc.dram_tensor("beta", [C], F32, kind="ExternalInput")
    gfwd_d = nc.dram_tensor("gfwd", [P, CB, GROUPS], F32, kind="ExternalInput")
    gbwd_d = nc.dram_tensor("gbwd", [GROUPS, CB, P], F32, kind="ExternalInput")
    y_d = nc.dram_tensor("y", [BB, C, N], F32, kind="ExternalOutput")

    WKEYS = ("q", "k", "v")

    with tile.TileContext(nc) as tc:
        with (
            tc.tile_pool(name="singles", bufs=1) as sg,
            tc.tile_pool(name="sbp", bufs=1) as sbp,
            tc.tile_pool(name="psp", bufs=1, space="PSUM") as psp,
            tc.tile_pool(name="drp", bufs=1, space="DRAM") as drp,
        ):
            xview = [xs_d[b].rearrange("(cb p) n -> p cb n", p=P) for b in range(BB)]
            xtview = [xt_d[b].rearrange("(nt p) c -> p nt c", p=P) for b in range(BB)]
            yview = [y_d[b].rearrange("(ob p) n -> p ob n", p=P) for b in range(BB)]
            wt_dram = {"q": wqt_d, "k": wkt_d, "v": wvt_d}
            bias_dram = {}
            st = [dict() for _ in range(BB)]  # per-batch tile state

            def emit_load_t(b, first=False):
                """xT halves (fp16). DMA only."""
                s = st[b]
                xta = sbp.tile([P, NTH, C], FP16, tag="xta", bufs=1, name=f"xta{b}")
                xtb = sbp.tile([P, NTH, C], FP16, tag="xtb", bufs=1, name=f"xtb{b}")
                s["xta"], s["xtb"] = xta, xtb
                for h in range(4):
                    nc.sync.dma_start(
                        xta[:, ts(h, 4), :], xtview[b][:, 4 * h : 4 * h + 4, :]
                    )
                if first:
                    emit_load_x(b)
                for h in range(4):
                    nc.sync.dma_start(
                        xtb[:, ts(h, 4), :],
                        xtview[b][:, NTH + 4 * h : NTH + 4 * h + 4, :],
                    )

            def emit_load_x(b):
                """x (bf16). DMA only."""
                s = st[b]
                xsb = sbp.tile([P, CB, N], BF16, tag="xsb", bufs=2, name=f"xsb{b}")
                s["xsb"] = xsb
                for cb in range(CB):
                    nc.sync.dma_start(xsb[:, cb, :], xview[b][:, cb, :])

            def emit_stats(b):
                """Per-channel [mean, E[x^2]] -> t. Split: 2 blocks on ACT
                (Copy/Square + accum), 2 on DVE bn_stats — halves the span."""
                s = st[b]
                xsb = s["xsb"]
                t = sbp.tile([P, CB, 2], F32, tag="t", bufs=2, name=f"t{b}")
                act_cbs = (0, 1)
                bn_cbs = (2, 3)
                stats = sbp.tile(
                    [P, 2, 8, 6], F32, tag="stats", bufs=2, name=f"st{b}"
                )
                mv = sbp.tile([P, 2, 2], F32, tag="mv", bufs=2, name=f"mv{b}")
                for cb in act_cbs:
                    sq = sbp.tile([P, N], F32, tag="sq", bufs=1, name=f"sq{b}{cb}")
                    s1 = sbp.tile([P, 1], F32, tag="s1", bufs=2, name=f"s1{b}{cb}")
                    s2 = sbp.tile([P, 1], F32, tag="s2", bufs=2, name=f"s2{b}{cb}")
                    nc.scalar.activation(
                        sq, xsb[:, cb, :], AF.Copy, accum_out=s1
                    )
                    nc.scalar.activation(
                        sq, xsb[:, cb, :], AF.Square, accum_out=s2
                    )
                    nc.vector.tensor_scalar_mul(t[:, cb, 0:1], s1, 1.0 / N)
                    nc.vector.tensor_scalar_mul(t[:, cb, 1:2], s2, 1.0 / N)
                for j2, cb in enumerate(bn_cbs):
                    for j in range(8):
                        nc.vector.bn_stats(
                            stats[:, j2, j, :], xsb[:, cb, ts(j, 512)]
                        )
                    nc.vector.bn_aggr(mv[:, j2, :], stats[:, j2, :, :])
                    nc.vector.tensor_mul(
                        t[:, cb, 1:2], mv[:, j2, 0:1], mv[:, j2, 0:1]
                    )
                    nc.vector.tensor_add(
                        t[:, cb, 1:2], t[:, cb, 1:2], mv[:, j2, 1:2]
                    )
                    nc.vector.tensor_copy(t[:, cb, 0:1], mv[:, j2, 0:1])
                s["t"] = t

            def emit_a2(b):
                """Group aggregation -> A/B affine; scaled weights; bias rows;
                rank-1 score terms; A-scaled x."""
                s = st[b]
                t = s["t"]
                pg = psp.tile([GROUPS, 2], F32, tag="work", bufs=4, name=f"pg{b}")
                for cb in range(CB):
                    nc.tensor.matmul(
                        pg, gfwd[:, cb, :], t[:, cb, :],
                        start=(cb == 0), stop=(cb == CB - 1),
                    )
                gs = sbp.tile([GROUPS, 2], F32, tag="gs", bufs=2, name=f"gs{b}")
                pgs = sbp.tile([GROUPS, 2], F32, tag="pgs", bufs=2, name=f"pgs{b}")
                nc.vector.tensor_copy(pgs, pg)
                vtmp = sbp.tile([GROUPS, 1], F32, tag="vtmp", bufs=2, name=f"vt{b}")
                nc.vector.tensor_mul(vtmp, pgs[:, 0:1], pgs[:, 0:1])
                nc.vector.tensor_tensor(vtmp, pgs[:, 1:2], vtmp, op=OP.subtract)
                nc.vector.tensor_copy(gs[:, 0:1], pgs[:, 0:1])
                nc.scalar.activation(gs[:, 1:2], vtmp, AF.Sqrt, bias=eps_g)
                nc.vector.reciprocal(gs[:, 1:2], gs[:, 1:2])

                cst = sbp.tile([P, CB, 2], F32, tag="cst", bufs=2, name=f"cs{b}")
                for cb in range(CB):
                    pc = psp.tile([P, 2], F32, tag="work", bufs=4, name=f"pc{b}_{cb}")
                    nc.tensor.matmul(pc, gbwd[:, cb, :], gs, start=True, stop=True)
                    nc.vector.tensor_copy(cst[:, cb, :], pc)

                A_ = sbp.tile([P, CB], F32, tag="A_", bufs=2, name=f"A{b}")
                Bb = sbp.tile([P, CB], FP16, tag="Bb", bufs=2, name=f"B{b}")
                tmpB = sbp.tile([P, CB], F32, tag="tmpB", bufs=2, name=f"tB{b}")
                nc.vector.tensor_mul(A_, cst[:, :, 1], gam)
                nc.vector.tensor_mul(tmpB, cst[:, :, 0], A_)
                nc.vector.tensor_tensor(Bb, bet, tmpB, op=OP.subtract)

                wq_p = sbp.tile([P, CB, C], FP16, tag="wq_p", bufs=1, name=f"wq{b}")
                wk_p = sbp.tile([P, CB, C], FP16, tag="wk_p", bufs=1, name=f"wk{b}")
                s["wq_p"], s["wk_p"] = wq_p, wk_p
                for wi, wsc in ((0, wq_p), (1, wk_p)):
                    for cb in range(CB):
                        nc.vector.tensor_scalar_mul(
                            wsc[:, cb, :], wall[:, wi, cb, :], A_[:, cb : cb + 1]
                        )
                # folded bias rows b'_w = W @ B + b_w  (bf16 rows for rank-1 MMs)
                rows = {}
                for wi, w in enumerate(WKEYS):
                    pb = psp.tile([1, C], F32, tag="work", bufs=4, name=f"pb{b}{w}")
                    for cb in range(CB):
                        nc.tensor.matmul(
                            pb, Bb[:, cb : cb + 1], wall[:, wi, cb, :],
                            start=(cb == 0), stop=(cb == CB - 1),
                        )
                    if w in ("q", "k"):
                        bfull = sbp.tile([1, C], BF16, tag=f"bf_{w}", bufs=2,
                                         name=f"bf{b}{w}")
                        nc.vector.tensor_add(bfull, pb, bias_dram[w])
                        rows[w] = bfull
                    else:
                        bfull = sbp.tile([1, C], F32, tag="bf_v", bufs=2,
                                         name=f"bf{b}{w}")
                        nc.vector.tensor_add(bfull, pb, bias_dram[w])
                        scr = drp.tile([C], F32, name=f"scr{b}{w}")
                        nc.sync.dma_start(scr.rearrange("(a c) -> a c", a=1), bfull)
                        bvb = sbp.tile([P, CB], F32, tag="bvb", bufs=2,
                                       name=f"bvb{b}")
                        nc.sync.dma_start(
                            bvb, scr.rearrange("(cb p) -> p cb", p=P)
                        )
                        bvbh = sbp.tile([P, CB], FP16, tag="bvbh", bufs=2,
                                        name=f"bvbh{b}")
                        nc.vector.tensor_copy(bvbh, bvb)
                        s["bvbh"] = bvbh
                s["bq_row"], s["bk_row"] = rows["q"], rows["k"]
                # rank-1 terms: u = WqA sx, w2 = WkA sx + N bk'
                sxc = sbp.tile([P, CB], FP16, tag="sxc", bufs=2, name=f"sx{b}")
                nc.vector.tensor_scalar_mul(sxc, t[:, :, 0], float(N))
                urow = sbp.tile([1, C], BF16, tag="urow", bufs=2, name=f"u{b}")
                wrow = sbp.tile([1, C], BF16, tag="wrow", bufs=2, name=f"w{b}")
                for wsc, dst in ((wq_p, urow), (wk_p, wrow)):
                    pu = psp.tile([1, C], F32, tag="work", bufs=4,
                                  name=f"pu{b}{dst.name}")
                    for cb in range(CB):
                        nc.tensor.matmul(
                            pu, sxc[:, cb : cb + 1], wsc[:, cb, :],
                            start=(cb == 0), stop=(cb == CB - 1),
                        )
                    nc.vector.tensor_copy(dst, pu)
                w2row = sbp.tile([1, C], BF16, tag="w2row", bufs=2, name=f"w2{b}")
                nc.vector.scalar_tensor_tensor(
                    w2row, rows["k"], float(N), wrow, op0=OP.mult, op1=OP.add
                )
                s["urow"], s["w2row"] = urow, w2row
                s["A_"] = A_

            def emit_xx(b):
                """Gram matrix XX = x x^T from fp16 xT tiles. Symmetric: only
                blocks j>=i computed; lower triangle mirrored by DMA transpose."""
                s = st[b]
                xta, xtb = s["xta"], s["xtb"]
                xxps = [
                    psp.tile([P, C - P * i], F32, tag="scores", bufs=4,
                             name=f"xx{b}_{i}")
                    for i in range(CB)
                ]
                for nt in range(NT):
                    src = xta if nt < NTH else xtb
                    idx = nt % NTH
                    for i in range(CB):
                        nc.tensor.matmul(
                            xxps[i], src[:, idx, ts(i, P)], src[:, idx, P * i :],
                            start=(nt == 0), stop=(nt == NT - 1),
                        )
                xxsb = sbp.tile([P, CB, C], FP16, tag="xxsb", bufs=1, name=f"xxs{b}")
                s["xxsb"] = xxsb
                for i in range(CB):
                    nc.vector.tensor_copy(xxsb[:, i, P * i :], xxps[i])
                for i in range(CB):
                    for j in range(i + 1, CB):
                        nc.sync.dma_start(
                            xxsb[:, j, ts(i, P)],
                            xxsb[:, i, ts(j, P)],
                            transpose=True,
                        )

            def emit_m1(b):
                """M1 = XX @ (WqA)^T  [e, c]. eb descending so the first groups
                read direct (upper-triangle) blocks while mirrors land."""
                s = st[b]
                xxsb, wq_p = s["xxsb"], s["wq_p"]
                m1sb = sbp.tile([P, CB, C], FP16, tag="m1sb", bufs=1, name=f"m1{b}")
                s["m1sb"] = m1sb
                for eb in reversed(range(CB)):
                    m1ps = psp.tile([P, C], F32, tag="work", bufs=4,
                                    name=f"m1p{b}_{eb}")
                    for fb in range(CB):
                        nc.tensor.matmul(
                            m1ps, xxsb[:, fb, ts(eb, P)], wq_p[:, fb, :],
                            start=(fb == 0), stop=(fb == CB - 1),
                        )
                    nc.vector.tensor_copy(m1sb[:, eb, :], m1ps)

            def emit_scores(b):
                """scores = M1^T (WkA)^T + rank-1 bias terms."""
                s = st[b]
                m1sb, wk_p = s["m1sb"], s["wk_p"]
                urow, w2row = s["urow"], s["w2row"]
                bq_row, bk_row = s["bq_row"], s["bk_row"]
                scores = [
                    psp.tile([P, C], F32, tag="scores", bufs=4, name=f"sc{b}_{cb}")
                    for cb in range(CB)
                ]
                s["scores"] = scores
                for cb in range(CB):
                    for eb in range(CB):
                        nc.tensor.matmul(
                            scores[cb], m1sb[:, eb, ts(cb, P)], wk_p[:, eb, :],
                            start=(eb == 0), stop=False,
                        )
                    nc.tensor.matmul(
                        scores[cb], urow[0:1, ts(cb, P)], bk_row,
                        start=False, stop=False,
                    )
                    nc.tensor.matmul(
                        scores[cb], bq_row[0:1, ts(cb, P)], w2row,
                        start=False, stop=True,
                    )

            def emit_softmax(b):
                s = st[b]
                scores = s["scores"]
                e_sb = sbp.tile([P, CB, C], BF16, tag="e", bufs=1, name=f"e{b}")
                rinv = sbp.tile([P, CB], F32, tag="rinv", bufs=1, name=f"ri{b}")
                s["e"], s["rinv"] = e_sb, rinv
                for cb in range(CB):
                    rs = sbp.tile([P, 1], F32, tag="rs", bufs=2, name=f"rs{b}{cb}")
                    nc.scalar.activation(
                        e_sb[:, cb, :], scores[cb], AF.Exp,
                        bias=0.0, scale=SCALE, accum_out=rs,
                    )
                    nc.vector.reciprocal(rinv[:, cb : cb + 1], rs)

            def emit_wor_ft(b):
                """WoR = rinv . Wo^T (bf16); Ft = e^T WoR [d, o];
                fbo = Ft^T bv' (per-o bias from folded v bias)."""
                s = st[b]
                e_sb, rinv, bvbh = s["e"], s["rinv"], s["bvbh"]
                wor = sbp.tile([P, CB, C], BF16, tag="wor", bufs=1, name=f"wo{b}")
                for cb in range(CB):
                    nc.vector.tensor_scalar_mul(
                        wor[:, cb, :], wot[:, cb, :], rinv[:, cb : cb + 1]
                    )
                ftsb = sbp.tile([P, CB, C], FP16, tag="ftsb", bufs=1, name=f"ft{b}")
                s["ftsb"] = ftsb
                for db in range(CB):
                    ftps = psp.tile([P, C], F32, tag="work", bufs=4,
                                    name=f"ftp{b}_{db}")
                    for cb in range(CB):
                        nc.tensor.matmul(
                            ftps, e_sb[:, cb, ts(db, P)], wor[:, cb, :],
                            start=(cb == 0), stop=(cb == CB - 1),
                        )
                    nc.vector.tensor_copy(ftsb[:, db, :], ftps)
                # fbo[o] = sum_d Ft[d,o] bv'[d]; fold into epilogue bias
                bobf = sbp.tile([P, CB], F32, tag="bobf", bufs=2, name=f"bo{b}")
                s["bobf"] = bobf
                fbo = sbp.tile([P, CB], F32, tag="fbo", bufs=2, name=f"fb{b}")
                for ob in range(CB):
                    fbps = psp.tile([P, 1], F32, tag="work", bufs=4,
                                    name=f"fbp{b}_{ob}")
                    for db in range(CB):
                        nc.tensor.matmul(
                            fbps, ftsb[:, db, ts(ob, P)], bvbh[:, db : db + 1],
                            start=(db == 0), stop=(db == CB - 1),
                        )
                    nc.vector.tensor_copy(fbo[:, ob : ob + 1], fbps)
                nc.vector.tensor_add(bobf, fbo, bob)

            def emit_fwt(b):
                """FWt = A . (Wv^T Ft)  [e, o] — A folded into the evac so the
                epilogue can consume raw bf16 x."""
                s = st[b]
                ftsb, A_ = s["ftsb"], s["A_"]
                fwsb = sbp.tile([P, CB, C], FP16, tag="fwsb", bufs=1, name=f"fw{b}")
                s["fwsb"] = fwsb
                for eb in range(CB):
                    fwps = psp.tile([P, C], F32, tag="work", bufs=4,
                                    name=f"fwp{b}_{eb}")
                    for db in range(CB):
                        nc.tensor.matmul(
                            fwps, wvnt[:, db, ts(eb, P)], ftsb[:, db, :],
                            start=(db == 0), stop=(db == CB - 1),
                        )
                    nc.vector.tensor_scalar_mul(
                        fwsb[:, eb, :], fwps, A_[:, eb : eb + 1]
                    )

            def emit_ef(b, n0=0, n1=NS):
                """out = (A.FWt)^T x + (bobf)1^T;  y = x + out."""
                s = st[b]
                fwsb, xsb, bobf = s["fwsb"], s["xsb"], s["bobf"]
                for nsl in range(n0, n1):
                    for ob in range(CB):
                        pf = psp.tile([P, NSL], F32, tag="work", bufs=4,
                                      name=f"pf{b}{nsl}{ob}")
                        for eb in range(CB):
                            nc.tensor.matmul(
                                pf, fwsb[:, eb, ts(ob, P)],
                                xsb[:, eb, ts(nsl, NSL)],
                                start=(eb == 0), stop=(eb == CB - 1),
                            )
                        yt = sbp.tile([P, NSL], F32, tag="yt", bufs=3,
                                      name=f"yt{b}{nsl}{ob}")
                        nc.vector.scalar_tensor_tensor(
                            yt, pf, bobf[:, ob : ob + 1],
                            xsb[:, ob, ts(nsl, NSL)],
                            op0=OP.add, op1=OP.add,
                        )
                        nc.gpsimd.dma_start(yview[b][:, ob, ts(nsl, NSL)], yt)

            # ---- prologue ----
            emit_load_t(0, first=True)
            # HAM warm-up: keep TensorE busy through the prologue so the Gram
            # phase starts at full clock. The dummy accumulator drains to DRAM
            # so the chain is not dead code.
            zsb = sg.tile([P, NSL], BF16, name="zsb")
            nc.gpsimd.memset(zsb, 0.0)
            pdum = psp.tile([P, NSL], F32, tag="work", bufs=4, name="pdum")
            for i in range(16):
                nc.tensor.matmul(
                    pdum, zsb[:, :P], zsb, start=(i == 0), stop=False
                )
            for cb in range(CB):
                nc.tensor.matmul(
                    pdum, st[0]["xsb"][:, cb, ts(0, P)], zsb,
                    start=False, stop=(cb == CB - 1),
                )
            dsb = sg.tile([1, 1], F32, name="dsb")
            nc.vector.tensor_copy(dsb, pdum[0:1, 0:1])
            dscr = drp.tile([1], F32, name="dscr")
            nc.sync.dma_start(dscr.rearrange("(a c) -> a c", a=1), dsb)
            # ---- constants, loaded once ----
            gfwd = sg.tile([P, CB, GROUPS], F32)
            nc.sync.dma_start(gfwd, gfwd_d[:])
            gbwd = sg.tile([GROUPS, CB, P], F32)
            nc.sync.dma_start(gbwd, gbwd_d[:])
            wall = sg.tile([P, 3, CB, C], FP16)
            for wi, w in enumerate(WKEYS):
                for cb in range(CB):
                    nc.sync.dma_start(wall[:, wi, cb, :], wt_dram[w][ts(cb, P), :])
            wvnt = sg.tile([P, CB, C], FP16)
            nc.sync.dma_start(wvnt, wvnt_d[:].rearrange("(cb p) e -> p cb e", p=P))
            wot = sg.tile([P, CB, C], FP16)
            nc.sync.dma_start(wot, wot_d[:].rearrange("(cb p) o -> p cb o", p=P))
            gam = sg.tile([P, CB], F32)
            nc.sync.dma_start(gam, gamma_d[:].rearrange("(cb p) -> p cb", p=P))
            bet = sg.tile([P, CB], F32)
            nc.sync.dma_start(bet, beta_d[:].rearrange("(cb p) -> p cb", p=P))
            bob = sg.tile([P, CB], F32)
            nc.sync.dma_start(bob, bo_d[:].rearrange("(cb p) -> p cb", p=P))
            bqv = sg.tile([1, C], F32)
            nc.sync.dma_start(bqv, bq_d[:].rearrange("(a c) -> a c", a=1))
            bkv = sg.tile([1, C], F32)
            nc.sync.dma_start(bkv, bk_d[:].rearrange("(a c) -> a c", a=1))
            bvv = sg.tile([1, C], F32)
            nc.sync.dma_start(bvv, bv_d[:].rearrange("(a c) -> a c", a=1))
            eps_g = sg.tile([GROUPS, 1], F32)
            nc.vector.memset(eps_g, EPS)
            bias_dram["q"], bias_dram["k"], bias_dram["v"] = bqv, bkv, bvv

            emit_stats(0)
            emit_a2(0)
            # ---- software-pipelined emission across the two batches ----
            emit_xx(0)
            emit_load_x(1)
            emit_m1(0)
            emit_scores(0)
            emit_stats(1)       # DVE/ACT; runs while PE is in XX(1)
            emit_softmax(0)
            emit_load_t(1)
            emit_xx(1)          # fills PE during batch-0 softmax
            emit_a2(1)
            emit_wor_ft(0)
            emit_fwt(0)
            emit_ef(0, 0, 6)
            emit_m1(1)
            emit_scores(1)
            emit_softmax(1)
            emit_ef(0, 6, 8)    # fills PE during batch-1 softmax
            emit_wor_ft(1)
            emit_fwt(1)
            emit_ef(1)

    nc.finalize()
    return nc


def _get_nc():
    if "nc" not in _NC_CACHE:
        _NC_CACHE["nc"] = _build_nc()
    return _NC_CACHE["nc"]


def _make_consts():
    gfwd = np.zeros((P, CB, GROUPS), np.float32)
    gbwd = np.zeros((GROUPS, CB, P), np.float32)
    for cb in range(CB):
        for p in range(P):
            g = (cb * P + p) // 16
            gfwd[p, cb, g] = 1.0 / 16.0
            gbwd[g, cb, p] = 1.0
    return gfwd, gbwd


def kernel(x, gamma, beta, Wq, bq, Wk, bk, Wv, bv, Wo, bo):
    global LAST_RESULT
    from concourse.bass_utils import run_bass_kernel_spmd

    import ml_dtypes

    BF = ml_dtypes.bfloat16
    H = np.float16
    x = np.ascontiguousarray(np.asarray(x, np.float32)).reshape(16, C, N)
    xbf = np.ascontiguousarray(x.astype(BF))
    xth = np.ascontiguousarray(np.transpose(x, (0, 2, 1)).astype(H))
    gfwd, gbwd = _make_consts()
    shared = {
        "wqt": np.ascontiguousarray(np.asarray(Wq, np.float32).T.astype(H)),
        "wkt": np.ascontiguousarray(np.asarray(Wk, np.float32).T.astype(H)),
        "wvt": np.ascontiguousarray(np.asarray(Wv, np.float32).T.astype(H)),
        "wvnt": np.ascontiguousarray(np.asarray(Wv, np.float32).astype(H)),
        "wot": np.ascontiguousarray(np.asarray(Wo, np.float32).T.astype(H)),
        "bq": np.ascontiguousarray(np.asarray(bq, np.float32)),
        "bk": np.ascontiguousarray(np.asarray(bk, np.float32)),
        "bv": np.ascontiguousarray(np.asarray(bv, np.float32)),
        "bo": np.ascontiguousarray(np.asarray(bo, np.float32)),
        "gamma": np.ascontiguousarray(np.asarray(gamma, np.float32)),
        "beta": np.ascontiguousarray(np.asarray(beta, np.float32)),
        "gfwd": gfwd,
        "gbwd": gbwd,
    }
    in_maps = [
        dict(
            shared,
            xs=np.ascontiguousarray(xbf[BB * i : BB * (i + 1)]),
            xt=np.ascontiguousarray(xth[BB * i : BB * (i + 1)]),
        )
        for i in range(8)
    ]
    nc = _get_nc()
    import os

    trace = os.environ.get("KERNEL_TRACE") == "1"
    res = run_bass_kernel_spmd(nc, in_maps, core_ids=list(range(8)), trace=trace)
    LAST_RESULT = res
    y = np.concatenate([r["y"] for r in res.results], axis=0)
    return y.reshape(16, C, 64, 64)
